# revision 2
# baseline (speedup 1.0000x reference)
"""Supervised contrastive loss kernel for Trainium2 (8 NeuronCores, Bass/Tile).

Row-parallel sharding: core c owns rows [c*1024, (c+1)*1024) of the
8192x1024 feature matrix.  Each core:
  1. loads its raw row block, L2-normalizes rows (scale folded with
     1/sqrt(temperature)) into bf16 `g` (row-major),
  2. computes a partial class-sum matrix C_part = g_c^T @ onehot_c and
     AllReduces it (C[d,t] = sum_j g[j,d] * [type_j == t]),
  3. PE-transposes its block to gT_c [D, 1024] and AllGathers the blocks so
     every core holds gT [D, 8192] resident in SBUF (bf16, 16 MB),
  4. computes its [1024, 8192] similarity block with bf16 matmuls
     (fp32 PSUM accumulation), masks its diagonal element via a per-core
     additive -1e9 mask input (pure data-driven, keeps the program SPMD),
     applies exp in-place on PSUM with a fused per-row accumulation
     (denominators), and gets the positive-pair sums from P = G @ C
     (select own-type column via onehot, subtract the diagonal's 1/T),
  5. computes ln(exp(pos_mean)/denom + EPS) per row and writes [128, 8].
Host combines: loss = -sum(valid rows) / count(valid rows).
"""

import numpy as np
import ml_dtypes

import concourse.bass as bass
import concourse.bacc as bacc
import concourse.mybir as mybir
from concourse import tile
from concourse.bass_utils import run_bass_kernel_spmd

N, D, NT, NC = 8192, 1024, 32, 8
R = N // NC          # rows per core
RT = R // 128        # row tiles per core (m-tiles)
KT = D // 128        # contraction chunks
NTILE = N // 1024    # 1024-wide column chunks
T = 0.07
EPS = 1e-10
NEG = -1.0e9
WW = 7 * 128 + N     # width of the shifted diagonal-mask input

F32 = mybir.dt.float32
BF16 = mybir.dt.bfloat16
BF16_NP = ml_dtypes.bfloat16


def build_program():
    nc = bacc.Bacc(None, target_bir_lowering=False, debug=False)
    feat = nc.dram_tensor("feat_rows", [R, D], F32, kind="ExternalInput")
    # onehot pre-arranged on host to [partition, rt*NT]
    ohr = nc.dram_tensor("oh_rows", [128, RT * NT], BF16, kind="ExternalInput")
    wmask = nc.dram_tensor("wmask", [128, WW], BF16, kind="ExternalInput")
    ident = nc.dram_tensor("ident", [128, 128], BF16, kind="ExternalInput")
    deno = nc.dram_tensor("den_o", [128, RT], F32, kind="ExternalOutput")
    poso = nc.dram_tensor("pos_o", [128, RT], F32, kind="ExternalOutput")

    AX = mybir.AxisListType.X
    MUL = mybir.AluOpType.mult
    ADD = mybir.AluOpType.add
    AF = mybir.ActivationFunctionType

    with tile.TileContext(nc) as tc:
        with (
            tc.tile_pool(name="dram", bufs=1, space="DRAM") as dpool,
            tc.tile_pool(name="big", bufs=1) as big,
            tc.tile_pool(name="work", bufs=2) as work,
            tc.tile_pool(name="stats", bufs=1) as stats,
            tc.tile_pool(name="psum", bufs=3, space="PSUM") as psum,
            tc.tile_pool(name="psmall", bufs=2, space="PSUM") as psmall,
        ):
            # internal DRAM (collective bounce buffers)
            gTc_d = dpool.tile([D, R], BF16, tag="gTc")
            gT_all = dpool.tile([NC, D, R], BF16, tag="gTall", addr_space="Shared")
            Cpart_d = dpool.tile([128, KT * NT], F32, tag="Cpart")
            Call_d = dpool.tile([128, KT * NT], F32, tag="Call", addr_space="Shared")

            # resident SBUF
            gT = big.tile([128, KT, N], BF16, tag="gT")      # gathered fT
            gTo = big.tile([128, KT, R], BF16, tag="gTo")    # own block fT
            grow = big.tile([128, RT, D], BF16, tag="grow")  # own rows
            W = big.tile([128, WW], BF16, tag="W")
            oh = big.tile([128, RT * NT], BF16, tag="oh")
            idn = big.tile([128, 128], BF16, tag="idn")
            Cbf = big.tile([128, KT * NT], BF16, tag="Cbf")

            ssq = stats.tile([128, RT], F32, tag="ssq")
            nrm = stats.tile([128, RT], F32, tag="nrm")
            scl = stats.tile([128, RT], F32, tag="scl")
            den = stats.tile([128, RT], F32, tag="den")
            pos = stats.tile([128, RT], F32, tag="pos")
            Cst = stats.tile([128, KT * NT], F32, tag="Cst")

            # constants / parameters in
            nc.gpsimd.dma_start(idn[:, :], ident[:, :])
            nc.gpsimd.dma_start(oh[:, :], ohr[:, :])
            nc.gpsimd.dma_start(W[:, :], wmask[:, :])

            # ---- phase 1: row sum-of-squares ----
            for rt in range(RT):
                raw = work.tile([128, D], F32, tag="raw")
                nc.gpsimd.dma_start(raw[:, :], feat[rt * 128 : (rt + 1) * 128, :])
                dump = psum.tile([128, 1024], F32, tag="sim")
                nc.vector.tensor_mul(dump[:, :], raw[:, :], raw[:, :])
                nc.vector.reduce_sum(ssq[:, rt : rt + 1], dump[:, :], axis=AX)
            # s = 1 / max(sqrt(T * ssq), sqrt(T)*1e-12)
            nc.scalar.activation(nrm[:, :], ssq[:, :], AF.Sqrt, scale=float(T))
            nc.vector.tensor_scalar_max(nrm[:, :], nrm[:, :], float(np.sqrt(T) * 1e-12))
            nc.vector.reciprocal(scl[:, :], nrm[:, :])

            # ---- phase 1b: g = raw * s  (bf16, row-major) ----
            for rt in range(RT):
                raw = work.tile([128, D], F32, tag="raw")
                nc.gpsimd.dma_start(raw[:, :], feat[rt * 128 : (rt + 1) * 128, :])
                nc.vector.tensor_scalar_mul(
                    grow[:, rt, :], raw[:, :], scl[:, rt : rt + 1]
                )

            # ---- phase 2: partial class sums C_part[d, t] ----
            for dt in range(KT):
                cps = psmall.tile([128, 128], F32, tag="small")
                for jt in range(RT):
                    nc.tensor.matmul(
                        cps[:, 0:NT],
                        grow[:, jt, dt * 128 : (dt + 1) * 128],
                        oh[:, jt * NT : (jt + 1) * NT],
                        start=(jt == 0),
                        stop=(jt == RT - 1),
                    )
                nc.vector.tensor_copy(Cst[:, dt * NT : (dt + 1) * NT], cps[:, 0:NT])
            nc.gpsimd.dma_start(Cpart_d[:, :], Cst[:, :])
            nc.gpsimd.collective_compute(
                "AllReduce",
                ADD,
                replica_groups=[list(range(NC))],
                ins=[Cpart_d.opt()],
                outs=[Call_d.opt()],
            )
            nc.gpsimd.dma_start(Cst[:, :], Call_d[:, :])
            nc.vector.tensor_copy(Cbf[:, :], Cst[:, :])

            # ---- phase 3: transpose own block (PE) and publish ----
            for dt in range(KT):
                for jt in range(RT):
                    tp = psmall.tile([128, 128], BF16, tag="small")
                    nc.tensor.transpose(
                        tp[:, :], grow[:, jt, dt * 128 : (dt + 1) * 128], idn[:, :]
                    )
                    nc.vector.tensor_copy(gTo[:, dt, jt * 128 : (jt + 1) * 128], tp[:, :])
                nc.gpsimd.dma_start(gTc_d[dt * 128 : (dt + 1) * 128, :], gTo[:, dt, :])

            # ---- phase 4: AllGather transposed blocks; load resident gT ----
            nc.gpsimd.collective_compute(
                "AllGather",
                mybir.AluOpType.bypass,
                replica_groups=[list(range(NC))],
                ins=[gTc_d.opt()],
                outs=[gT_all.opt()],
            )
            for kt in range(KT):
                for r in range(NC):
                    nc.gpsimd.dma_start(
                        gT[:, kt, r * R : (r + 1) * R],
                        gT_all[r, kt * 128 : (kt + 1) * 128, :],
                    )

            # ---- phase 5: main loop over 8 m-tiles x 8 column chunks ----
            for mt in range(RT):
                acc = work.tile([128, NTILE], F32, tag="acc")
                for nt in range(NTILE):
                    sp = psum.tile([128, 1024], F32, tag="sim")
                    for half in range(2):
                        o = nt * 1024 + half * 512
                        for kt in range(KT):
                            nc.tensor.matmul(
                                sp[:, half * 512 : (half + 1) * 512],
                                gTo[:, kt, mt * 128 : (mt + 1) * 128],
                                gT[:, kt, o : o + 512],
                                start=(kt == 0),
                                stop=(kt == KT - 1),
                            )
                    # additive diag mask (nonzero only where this core's
                    # diagonal block lands; pure input data)
                    woff = 896 - 128 * mt + 1024 * nt
                    nc.vector.tensor_add(sp[:, :], sp[:, :], W[:, woff : woff + 1024])
                    # exp in place on PSUM with fused row-sum
                    nc.scalar.activation(
                        sp[:, :], sp[:, :], AF.Exp, accum_out=acc[:, nt : nt + 1]
                    )
                nc.vector.reduce_sum(den[:, mt : mt + 1], acc[:, :], axis=AX)

                # positives: P = G @ C ; pos_sum = sum_t P*oh - 1/T
                pp = psmall.tile([128, 128], F32, tag="small")
                for kt in range(KT):
                    nc.tensor.matmul(
                        pp[:, 0:NT],
                        gTo[:, kt, mt * 128 : (mt + 1) * 128],
                        Cbf[:, kt * NT : (kt + 1) * NT],
                        start=(kt == 0),
                        stop=(kt == KT - 1),
                    )
                scr = work.tile([128, NT], F32, tag="scr")
                nc.vector.tensor_mul(scr[:, :], pp[:, 0:NT], oh[:, mt * NT : (mt + 1) * NT])
                nc.vector.reduce_sum(pos[:, mt : mt + 1], scr[:, :], axis=AX)

            # ---- epilogue: per-row stats out (host computes the log form) ----
            nc.gpsimd.dma_start(deno[:, :], den[:, :])
            nc.gpsimd.dma_start(poso[:, :], pos[:, :])

    # Bacc.compile() splits multi-wait instructions into event semaphores
    # (HW allows 1 wait per instruction), inserts ACT table loads, and
    # populates .instr bytes for extended-ISA instructions.
    nc.compile()
    return nc


_NC_CACHE = None


def _get_program():
    global _NC_CACHE
    if _NC_CACHE is None:
        _NC_CACHE = build_program()
    return _NC_CACHE


def kernel(features, element_types):
    f = np.ascontiguousarray(np.asarray(features), dtype=np.float32)
    t = np.asarray(element_types).astype(np.int64)
    assert f.shape == (N, D) and t.shape == (N,)

    hist = np.bincount(t, minlength=NT)
    cnt = hist[t] - 1
    valid = cnt > 0
    inv = (1.0 / np.maximum(cnt, 1)).astype(np.float32)
    OH = (t[:, None] == np.arange(NT)[None, :]).astype(BF16_NP)
    identity = np.eye(128, dtype=BF16_NP)

    in_maps = []
    for c in range(NC):
        Wm = np.zeros((128, WW), BF16_NP)
        Wm[np.arange(128), 896 + R * c + np.arange(128)] = BF16_NP(NEG)
        rows = slice(c * R, (c + 1) * R)
        # [R, NT] -> [128, RT*NT]: oh_pm[p, rt*NT+t] = OH[c*R + rt*128 + p, t]
        oh_pm = np.ascontiguousarray(
            OH[rows].reshape(RT, 128, NT).transpose(1, 0, 2).reshape(128, RT * NT)
        )
        in_maps.append(
            {
                "feat_rows": np.ascontiguousarray(f[rows]),
                "oh_rows": oh_pm,
                "wmask": Wm,
                "ident": identity,
            }
        )

    global _last_in_maps
    _last_in_maps = in_maps
    nc = _get_program()
    res = run_bass_kernel_spmd(nc, in_maps, list(range(NC))).results

    total = 0.0
    vc = int(valid.sum())
    for c in range(NC):
        den = np.asarray(res[c]["den_o"], dtype=np.float64).T.ravel()  # [1024]
        pos = np.asarray(res[c]["pos_o"], dtype=np.float64).T.ravel()
        rows = slice(c * R, (c + 1) * R)
        pm = (pos - 1.0 / T) * inv[rows].astype(np.float64)
        loss = -np.log(np.exp(pm) / den + EPS)
        total += float((loss * valid[rows]).sum())
    out = total / vc if vc > 0 else 0.0
    return np.float32(out)



# revision 4
# speedup vs baseline: 3.6836x; 3.6836x over previous
"""Supervised contrastive loss kernel for Trainium2 (8 NeuronCores, Bass/Tile).

Row-parallel sharding with host-side input staging: core c owns rows
[c*1024, (c+1)*1024) of the 8192x1024 feature matrix.  The host
L2-normalizes rows (scale folded with 1/sqrt(temperature)), transposes to
gT [D, N], quantizes to fp8e4m3 (x16 scaling for range), and hands every
core the full gT in block-rotated order (own 1024-column block first) --
replicated sharding of the column dimension, so no device collective or
on-device transpose is needed.

Each core computes its [1024, 8192] similarity block with fp8 DoubleRow
matmuls (2 fp8 weights per PE cell => K=256 per instruction, fp32 PSUM
accumulation), masks its own diagonal via a small additive -1e9 window
input (SPMD-uniform thanks to the block rotation), applies exp in place on
PSUM with a fused per-row accumulation (denominator partial sums), and
computes P = G @ C (class-sum matrix, also fp8) for the positive-pair
sums.  Outputs per core: den [128, 8] and P [128, 8*32].

Host combines: pos_i = P[i, type_i]/16 - 1/T, then the standard
log-form loss reduction.
"""

import numpy as np
import ml_dtypes

import concourse.bass as bass
import concourse.bacc as bacc
import concourse.mybir as mybir
from concourse import tile
from concourse.bass_utils import run_bass_kernel_spmd

N, D, NT, NC = 8192, 1024, 32, 8
R = N // NC          # rows per core
RT = R // 128        # row tiles per core (m-tiles)
KT = D // 128        # contraction chunks
T = 0.07
EPS = 1e-10
NEG = -1.0e9
SC = 16.0            # fp8 pre-scale per operand (PSUM carries SC^2 * sim)
WWIN = 1408          # width of the shifted diagonal-mask input

F32 = mybir.dt.float32
BF16 = mybir.dt.bfloat16
F8 = mybir.dt.float8e4
BF16_NP = ml_dtypes.bfloat16
F8_NP = ml_dtypes.float8_e4m3


def build_program():
    nc = bacc.Bacc(None, target_bir_lowering=False, debug=False)
    gx = nc.dram_tensor("gx", [128, NC * KT * 1024], F8, kind="ExternalInput")
    cc8 = nc.dram_tensor("cc8", [128, KT * NT], F8, kind="ExternalInput")
    wm = nc.dram_tensor("wm", [128, WWIN], BF16, kind="ExternalInput")
    deno = nc.dram_tensor("den_o", [128, RT], F32, kind="ExternalOutput")
    poso = nc.dram_tensor("pos_o", [128, RT * NT], F32, kind="ExternalOutput")

    AX = mybir.AxisListType.X
    AF = mybir.ActivationFunctionType
    DR = mybir.MatmulPerfMode.DoubleRow

    with tile.TileContext(nc) as tc:
        with (
            tc.tile_pool(name="big", bufs=1) as big,
            tc.tile_pool(name="stats", bufs=1) as stats,
            tc.tile_pool(name="psum", bufs=4, space="PSUM") as psum,
            tc.tile_pool(name="psmall", bufs=2, space="PSUM") as psmall,
        ):
            gsb = [
                big.tile([128, KT, 1024], F8, tag=f"g{d}", name=f"g{d}")
                for d in range(NC)
            ]
            csb = big.tile([128, KT * NT], F8, tag="cc")
            wsb = big.tile([128, WWIN], BF16, tag="wm")
            acc = stats.tile([128, RT, 2 * NC], F32, tag="acc")
            den = stats.tile([128, RT], F32, tag="den")
            pos = stats.tile([128, RT * NT], F32, tag="pos")

            nc.gpsimd.dma_start(csb[:, :], cc8[:, :])
            nc.gpsimd.dma_start(wsb[:, :], wm[:, :])
            for d in range(NC):
                nc.gpsimd.dma_start(
                    gsb[d][:, :, :], gx[:, d * KT * 1024 : (d + 1) * KT * 1024]
                )

            for mt in range(RT):
                ms = slice(mt * 128, (mt + 1) * 128)
                for d in range(NC):
                    for h in range(2):
                        sp = psum.tile([128, 512], F32, tag="sim")
                        for kp in range(KT // 2):
                            nc.tensor.matmul(
                                sp[:, :],
                                gsb[0][:, 2 * kp : 2 * kp + 2, ms],
                                gsb[d][:, 2 * kp : 2 * kp + 2, h * 512 : (h + 1) * 512],
                                start=(kp == 0),
                                stop=(kp == KT // 2 - 1),
                                perf_mode=DR,
                            )
                        if d == 0 and h == (mt // 4):
                            woff = 896 - 128 * mt + 512 * h
                            nc.vector.tensor_add(
                                sp[:, :], sp[:, :], wsb[:, woff : woff + 512]
                            )
                        ci = d * 2 + h
                        nc.scalar.activation(
                            sp[:, :], sp[:, :], AF.Exp,
                            scale=1.0 / (SC * SC),
                            accum_out=acc[:, mt, ci : ci + 1],
                        )
                nc.vector.reduce_sum(den[:, mt : mt + 1], acc[:, mt, :], axis=AX)

                pp = psmall.tile([128, NT], F32, tag="pp")
                for kt in range(KT):
                    nc.tensor.matmul(
                        pp[:, :],
                        gsb[0][:, kt, ms],
                        csb[:, kt * NT : (kt + 1) * NT],
                        start=(kt == 0),
                        stop=(kt == KT - 1),
                    )
                nc.vector.tensor_copy(pos[:, mt * NT : (mt + 1) * NT], pp[:, :])

            nc.gpsimd.dma_start(deno[:, :], den[:, :])
            nc.gpsimd.dma_start(poso[:, :], pos[:, :])

    nc.compile()
    return nc


_NC_CACHE = None
_last_in_maps = None


def _get_program():
    global _NC_CACHE
    if _NC_CACHE is None:
        _NC_CACHE = build_program()
    return _NC_CACHE


def _build_in_maps(f, t):
    """f: [N, D] float32 features, t: [N] int64 types -> per-core input dicts."""
    nrm = np.maximum(np.sqrt((f.astype(np.float64) ** 2).sum(axis=1)), 1e-12)
    g = (f / nrm[:, None].astype(np.float32)) * np.float32(1.0 / np.sqrt(T))
    g8 = (g * np.float32(SC)).astype(F8_NP)            # [N, D] fp8, x16

    # per-block [128, KT*1024]: B[b][p, kt*1024 + j] = g8.T[kt*128+p, b*1024+j]
    gT = np.ascontiguousarray(g8.T)                     # [D, N]
    Gb = gT.reshape(KT, 128, NC, 1024)                  # [kt, p, b, j]
    blocks = [
        np.ascontiguousarray(Gb[:, :, b, :].transpose(1, 0, 2).reshape(128, KT * 1024))
        for b in range(NC)
    ]

    OH = (t[:, None] == np.arange(NT)[None, :]).astype(np.float32)
    C = g.T @ OH                                        # [D, NT] fp32 (class sums)
    c8 = np.ascontiguousarray(
        C.reshape(KT, 128, NT).transpose(1, 0, 2).reshape(128, KT * NT)
    ).astype(F8_NP)

    wmask = np.zeros((128, WWIN), BF16_NP)
    wmask[np.arange(128), 896 + np.arange(128)] = BF16_NP(NEG)

    in_maps = []
    for c in range(NC):
        in_maps.append(
            {
                "gx": np.concatenate([blocks[(c + d) % NC] for d in range(NC)], axis=1),
                "cc8": c8,
                "wm": wmask,
            }
        )
    return in_maps


def kernel(features, element_types):
    f = np.ascontiguousarray(np.asarray(features), dtype=np.float32)
    t = np.asarray(element_types).astype(np.int64)
    assert f.shape == (N, D) and t.shape == (N,)

    hist = np.bincount(t, minlength=NT)
    cnt = hist[t] - 1
    valid = cnt > 0
    in_maps = _build_in_maps(f, t)
    global _last_in_maps
    _last_in_maps = in_maps

    nc = _get_program()
    res = run_bass_kernel_spmd(nc, in_maps, list(range(NC))).results

    total = 0.0
    vc = int(valid.sum())
    for c in range(NC):
        den = np.asarray(res[c]["den_o"], dtype=np.float64).T.ravel()   # [1024]
        P = (
            np.asarray(res[c]["pos_o"], dtype=np.float64)
            .reshape(128, RT, NT)
            .transpose(1, 0, 2)
            .reshape(R, NT)
            / SC
        )
        rows = slice(c * R, (c + 1) * R)
        pos_sum = P[np.arange(R), t[rows]] - 1.0 / T
        pm = pos_sum / np.maximum(cnt[rows], 1)
        loss = -np.log(np.exp(pm) / den + EPS)
        total += float((loss * valid[rows]).sum())
    out = total / vc if vc > 0 else 0.0
    return np.float32(out)
